# revision 2
# baseline (speedup 1.0000x reference)
"""Causal multi-head attention with RoPE on 8 Trainium2 NeuronCores.

Problem: B=2, N=2048, DIM=1024, H=16, DH=64, fp32.
Sharding: head-parallel — core c owns heads 2c, 2c+1 (columns c*128:(c+1)*128
of Wq/Wk/Wv, rows c*128:(c+1)*128 of Wo) for both batches. Each core computes
its partial output projection [DIM, B*N]; the host sums the 8 partials
(the "all-reduce") and adds the bias.

On-device pipeline (all matmul operands are float32r = 8e11m, full PE rate):
  phase 1: per 512-token chunk: QKV projections (K=1024 accum), RoPE fused
           from PSUM on DVE -> qT/kT resident [128, 4096]; V transposed via
           PE into [j, d] layout + ones column (for softmax sums).
  phase 2: per (batch, i-chunk, head): S^T = k q^T per 128-j-block with the
           causal mask added in-PSUM via an identity-x-mask matmul; exp on
           ACT -> fp32r; augmented V-matmul accumulates both O'^T (rows 0:64)
           and softmax sums (row 64); reciprocal + K=1 ones-matmul broadcast;
           normalize on DVE; fused Wo projection per token chunk.
"""
import numpy as np
from concourse import bacc
import concourse.mybir as mybir
from concourse.tile import TileContext
from concourse.bass_utils import run_bass_kernel_spmd

B, N, DIM, H, DH = 2, 2048, 1024, 16, 64
NCORES = 8
HPC = H // NCORES          # 2 heads per core
T = B * N                  # 4096 tokens
CHUNK = 512
NCH = T // CHUNK           # 8 token chunks
NCB = DIM // 128           # 8 contraction blocks
NINST = B * HPC            # 4 attention instances per core
NJB = N // 128             # 16 j-blocks per batch
NEG = -1e9

F32 = mybir.dt.float32
F32R = mybir.dt.float32r

_NC_CACHE = {}


def build():
    nc = bacc.Bacc()
    xTD = nc.dram_tensor("xT", [DIM, T], F32R, kind="ExternalInput")
    wqD = nc.dram_tensor("wq", [DIM, 128], F32R, kind="ExternalInput")
    wkD = nc.dram_tensor("wk", [DIM, 128], F32R, kind="ExternalInput")
    wvD = nc.dram_tensor("wv", [DIM, 128], F32R, kind="ExternalInput")
    woD = nc.dram_tensor("wo", [128, DIM], F32R, kind="ExternalInput")
    cosD = nc.dram_tensor("cosT", [DH, N], F32, kind="ExternalInput")
    sinsD = nc.dram_tensor("sinsT", [DH, N], F32, kind="ExternalInput")
    identD = nc.dram_tensor("identD", [128, 128], F32R, kind="ExternalInput")
    ident2D = nc.dram_tensor("ident2D", [128, DH], F32R, kind="ExternalInput")
    masksD = nc.dram_tensor("masksD", [128, 4, CHUNK], F32R, kind="ExternalInput")
    ones1D = nc.dram_tensor("ones1D", [1, DH], F32R, kind="ExternalInput")
    onesColD = nc.dram_tensor("onesColD", [128, NINST, NJB, 1], F32R,
                              kind="ExternalInput")
    outD = nc.dram_tensor("outT", [DIM, T], F32, kind="ExternalOutput")

    Exp = mybir.ActivationFunctionType.Exp

    with TileContext(nc) as tc:
        with (
            tc.tile_pool(name="const", bufs=1) as cp,
            tc.tile_pool(name="sb", bufs=2) as sb,
        ):
            ident = cp.tile([128, 128], F32R)
            ident2 = cp.tile([128, DH], F32R)
            nc.sync.dma_start(out=ident2, in_=ident2D[:])
            masks = cp.tile([128, 4, CHUNK], F32R)
            ones1 = cp.tile([1, DH], F32R)
            wq = cp.tile([128, NCB, 128], F32R)
            wk = cp.tile([128, NCB, 128], F32R)
            wv = cp.tile([128, NCB, 128], F32R)
            wo = cp.tile([128, NCB, 128], F32R)
            nc.sync.dma_start(out=ident, in_=identD[:])
            nc.sync.dma_start(out=masks, in_=masksD[:])
            nc.sync.dma_start(out=ones1, in_=ones1D[:])
            for t, d in ((wq, wqD), (wk, wkD), (wv, wvD)):
                nc.sync.dma_start(
                    out=t, in_=d[:].rearrange("(cb p) d -> p cb d", p=128))
            nc.sync.dma_start(
                out=wo, in_=woD[:].rearrange("p (db d) -> p db d", d=128))

            # cos/sin duplicated across the two heads (rows) and batches (cols)
            cos2 = cp.tile([128, T], F32)
            sins2 = cp.tile([128, T], F32)
            for hh in (0, 64):
                for bb in range(B):
                    nc.sync.dma_start(
                        out=cos2[hh:hh + 64, bb * N:(bb + 1) * N], in_=cosD[:])
                    nc.sync.dma_start(
                        out=sins2[hh:hh + 64, bb * N:(bb + 1) * N], in_=sinsD[:])

            qt = cp.tile([128, T], F32R)     # rows 0:64 head0, 64:128 head1
            kt = cp.tile([128, T], F32R)
            v_aug = cp.tile([128, NINST, NJB, DH + 1], F32R)
            nc.sync.dma_start(out=v_aug[:, :, :, DH:DH + 1], in_=onesColD[:])

            # ---------------- phase 1: QKV + RoPE + V transpose ----------
            with tc.tile_pool(name="ps1", bufs=1, space="PSUM") as ps1:
                for ch in range(NCH):
                    t0 = ch * CHUNK
                    xt = sb.tile([128, NCB, CHUNK], F32R, tag="xt", bufs=2)
                    nc.sync.dma_start(
                        out=xt,
                        in_=xTD[:, t0:t0 + CHUNK].rearrange(
                            "(cb p) n -> p cb n", p=128))
                    csl = slice(t0, t0 + CHUNK)
                    for which, W in (("q", wq), ("k", wk), ("v", wv)):
                        pp = ps1.tile([128, CHUNK], F32, tag="pqkv", bufs=2)
                        for cb in range(NCB):
                            nc.tensor.matmul(pp, W[:, cb, :], xt[:, cb, :],
                                             start=(cb == 0),
                                             stop=(cb == NCB - 1))
                        if which in ("q", "k"):
                            dst = qt if which == "q" else kt
                            tmp = sb.tile([128, CHUNK], F32, tag="tmp", bufs=2)
                            tmp2 = sb.tile([128, CHUNK], F32, tag="tmp2", bufs=2)
                            nc.vector.tensor_mul(tmp, pp, cos2[:, csl])
                            for hh in (0, 64):
                                a, bnd, c2 = hh, hh + 32, hh + 64
                                nc.vector.tensor_mul(
                                    tmp2[a:bnd, :], pp[bnd:c2, :],
                                    sins2[a:bnd, csl])
                                nc.vector.tensor_mul(
                                    tmp2[bnd:c2, :], pp[a:bnd, :],
                                    sins2[bnd:c2, csl])
                            nc.vector.tensor_add(dst[:, csl], tmp, tmp2)
                        else:
                            vtc = sb.tile([128, CHUNK], F32, tag="vtc", bufs=2)
                            nc.scalar.copy(vtc, pp)
                            bidx = ch // 4
                            for tb in range(4):
                                jb = (ch % 4) * 4 + tb
                                for h in range(HPC):
                                    pt = ps1.tile([128, DH], F32, tag="ptr",
                                                  bufs=2)
                                    nc.tensor.transpose(
                                        pt,
                                        vtc[h * 64:(h + 1) * 64,
                                            tb * 128:(tb + 1) * 128],
                                        ident2[h * 64:(h + 1) * 64, :].bitcast(F32))
                                    nc.vector.tensor_copy(
                                        v_aug[:, bidx * HPC + h, jb, 0:DH], pt)

            # ---------------- phase 2: attention + projection ------------
            with tc.tile_pool(name="ps2", bufs=1, space="PSUM") as ps2:
                for bidx in range(B):
                    for ch in range(N // CHUNK):      # i-chunk within batch
                        gcol = bidx * N + ch * CHUNK
                        njb = 4 * (ch + 1)
                        ot = sb.tile([128, CHUNK], F32R, tag="ot", bufs=2)
                        for h in range(HPC):
                            inst = bidx * HPC + h
                            qr = slice(h * 64, (h + 1) * 64)
                            po = ps2.tile([DH + 1, CHUNK], F32, tag="po",
                                          bufs=2)
                            for r0 in range(0, njb, 2):
                                pst = ps2.tile([128, 2, CHUNK], F32,
                                               tag="pst", bufs=2)
                                for idx in range(2):
                                    jb = r0 + idx
                                    jc = bidx * N + jb * 128
                                    diag = jb >= 4 * ch
                                    nc.tensor.matmul(
                                        pst[:, idx, :],
                                        kt[qr, jc:jc + 128],
                                        qt[qr, gcol:gcol + CHUNK],
                                        start=True, stop=not diag)
                                    if diag:
                                        nc.tensor.matmul(
                                            pst[:, idx, :], ident,
                                            masks[:, jb - 4 * ch, :],
                                            start=False, stop=True)
                                expt = sb.tile([128, 2, CHUNK], F32R,
                                               tag="expt", bufs=3)
                                nc.scalar.activation(expt, pst, Exp)
                                for idx in range(2):
                                    jb = r0 + idx
                                    nc.tensor.matmul(
                                        po, v_aug[:, inst, jb, :],
                                        expt[:, idx, :],
                                        start=(jb == 0), stop=(jb == njb - 1))
                            # normalize: 1/sum broadcast via K=1 matmul
                            rrow = sb.tile([1, CHUNK], F32R, tag="rrow", bufs=2)
                            with nc.allow_low_precision(reason="fp32r recip"):
                                nc.vector.reciprocal(rrow, po[DH:DH + 1, :])
                            pb = ps2.tile([DH, CHUNK], F32, tag="sm", bufs=2)
                            nc.tensor.matmul(pb, ones1, rrow,
                                             start=True, stop=True)
                            rb = sb.tile([DH, CHUNK], F32, tag="rb", bufs=2)
                            nc.vector.tensor_copy(rb, pb)
                            nc.vector.tensor_mul(ot[qr, :], po[0:DH, :], rb)
                        # fused output projection for this token chunk
                        for db in range(NCB):
                            ppr = ps2.tile([128, CHUNK], F32, tag="sm", bufs=2)
                            nc.tensor.matmul(ppr, wo[:, db, :], ot,
                                             start=True, stop=True)
                            osb = sb.tile([128, CHUNK], F32, tag="osb", bufs=3)
                            if db % 2 == 0:
                                nc.scalar.copy(osb, ppr)
                            else:
                                nc.vector.tensor_copy(osb, ppr)
                            nc.sync.dma_start(
                                out=outD[db * 128:(db + 1) * 128,
                                         gcol:gcol + CHUNK],
                                in_=osb)
    nc.compile()
    return nc


def _get_nc():
    if "nc" not in _NC_CACHE:
        _NC_CACHE["nc"] = build()
    return _NC_CACHE["nc"]


def make_in_maps(x, pos_emb, Wq, Wk, Wv, Wo):
    x = np.asarray(x, np.float32)
    pos_emb = np.asarray(pos_emb, np.float32)
    Wq = np.asarray(Wq, np.float32)
    Wk = np.asarray(Wk, np.float32)
    Wv = np.asarray(Wv, np.float32)
    Wo = np.asarray(Wo, np.float32)

    xT = np.ascontiguousarray(x.reshape(T, DIM).T)          # [DIM, T]
    cosT = np.ascontiguousarray(np.cos(pos_emb).T)          # [DH, N]
    sinT = np.sin(pos_emb).T
    sinsT = np.ascontiguousarray(
        np.concatenate([-sinT[0:32], sinT[32:64]], axis=0))  # sign-folded
    scale = np.float32(DH ** -0.5)

    ident = np.eye(128, dtype=np.float32)
    ident2 = np.tile(np.eye(DH, dtype=np.float32), (2, 1))
    jj = np.arange(128)[:, None]
    ii = np.arange(CHUNK)[None, :]
    masks = np.zeros((128, 4, CHUNK), np.float32)
    for r in range(4):
        masks[:, r, :] = np.where(r * 128 + jj <= ii, 0.0, NEG)
    ones1 = np.ones((1, DH), np.float32)
    ones_col = np.ones((128, NINST, NJB, 1), np.float32)

    in_maps = []
    for c in range(NCORES):
        cols = slice(c * 128, (c + 1) * 128)
        in_maps.append(dict(
            xT=xT,
            wq=np.ascontiguousarray(Wq[:, cols]) * scale,
            wk=np.ascontiguousarray(Wk[:, cols]),
            wv=np.ascontiguousarray(Wv[:, cols]),
            wo=np.ascontiguousarray(Wo[cols, :]),
            cosT=cosT, sinsT=sinsT, identD=ident, ident2D=ident2, masksD=masks,
            ones1D=ones1, onesColD=ones_col,
        ))
    return in_maps


def run(in_maps, trace=False, **kw):
    nc = _get_nc()
    return run_bass_kernel_spmd(nc, in_maps, list(range(NCORES)),
                                trace=trace, **kw)


def kernel(x, pos_emb, Wq, Wk, Wv, Wo, bo):
    in_maps = make_in_maps(x, pos_emb, Wq, Wk, Wv, Wo)
    res = run(in_maps)
    acc = np.zeros((DIM, T), np.float64)
    for c in range(NCORES):
        acc += res.results[c]["outT"]
    out = acc.T.reshape(B, N, DIM) + np.asarray(bo, np.float32)[None, None, :]
    return out.astype(np.float32)
